# revision 10
# baseline (speedup 1.0000x reference)
"""Trainium2 Bass kernel for nn_CombinedCS (FISTA compressed-sensing recon).

Self-contained: hardcodes shapes (B=16, H=W=320), shards batch over 8 cores
(2 images per core), runs the full 15-iteration FISTA loop SBUF-resident.

Math (validated vs reference in numpy, sim rel_err ~4e-3):
  - centered 2D FFT as two PE matmul stages against the DFT matrix
    (transpose-free: data stationary, F^T moving), fp16 operands
  - the k-space residual and gradient step are folded into the PE
    accumulations: fwd stage2 also adds -y (identity matmul), inverse
    stage2 uses negated G so PSUM = -g, then identity-matmuls add z and
    c0 = ifft2(mask*y), so PSUM = z - g + c0 = z_step directly
  - TV prox (5 Chambolle iters): h-direction div/grad as PE left-mults;
    w-direction via shifted views with zero guard columns; normalization
    clamp fused into the rescale via STT(min, mult)
  - 3-level Haar DWT as a CORRECTION: x_new = x_tv + W^T(soft(Wx)-Wx);
    the correction is bounded by the threshold so fp16 reconstruction
    is harmless and the image never round-trips through fp16

Scheduling: every matmul group targets a single-bank PSUM tile from an
8-deep ring pool and is consumed per-M-tile immediately; emission of
image i's TV/DWT phase is interleaved chunk-by-chunk with image 1-i's
FFT phase so the PE stays continuously fed (p-state) while DVE/ACT/
GPSIMD grind elementwise work.
"""
import math
import os

import numpy as np

H = W = 320
B = 16
NCORES = 8
IMGS = B // NCORES  # 2
LAM_TV = 0.005
LAM_WAV = 0.005
TAU = 0.25
TV_ITERS = 5
LEVELS = 3
MAX_ITER = int(os.environ.get("CS_ITERS", "15"))
SKIP_TV = os.environ.get("CS_SKIP_TV", "0") == "1"
SKIP_DWT = os.environ.get("CS_SKIP_DWT", "0") == "1"
S2 = math.sqrt(2.0)

P6D = {
    0: [(0, 128, 0, 0, 128), (0, 128, 1, 128, 256), (0, 64, 2, 256, 320)],
    1: [(0, 128, 3, 0, 128), (0, 128, 4, 128, 256), (0, 64, 5, 256, 320)],
}
P6C = [(0, 128, 0, 0, 128), (0, 128, 1, 128, 256), (0, 64, 2, 256, 320)]
L2D = {
    0: [(0, 128, 0, 0, 128), (0, 32, 1, 128, 160)],
    1: [(0, 128, 2, 0, 128), (0, 32, 3, 128, 160)],
}
L2C = [(0, 128, 0, 0, 128), (0, 32, 1, 128, 160)]
L3D = {
    0: [(0, 64, 0, 0, 64), (0, 16, 1, 64, 80)],
    1: [(0, 64, 2, 0, 64), (0, 16, 3, 64, 80)],
}
L3C = [(0, 64, 0, 0, 64), (0, 16, 1, 64, 80)]


def _dft_mats():
    I = np.eye(H, dtype=np.complex128)
    F = np.fft.fftshift(
        np.fft.fft(np.fft.ifftshift(I, axes=0), axis=0, norm="ortho"), axes=0
    )
    G = np.conj(F).T
    return F, G


def _tv_mats():
    Dd = np.zeros((H, H))
    Dd[0, 0] = 1.0
    for h in range(1, H - 1):
        Dd[h, h] = 1.0
        Dd[h, h - 1] = -1.0
    Dd[H - 1, H - 2] = -1.0
    Dg = np.zeros((H, H))
    for h in range(H - 1):
        Dg[h, h] = -1.0
        Dg[h, h + 1] = 1.0
    return Dd, Dg


def _haar_mat(n):
    Wm = np.zeros((n, n))
    hn = n // 2
    c = 1.0 / S2
    for i in range(hn):
        Wm[i, 2 * i] = c
        Wm[i, 2 * i + 1] = c
        Wm[hn + i, 2 * i] = c
        Wm[hn + i, 2 * i + 1] = -c
    return Wm


def _momentum_coeffs():
    t = 1.0
    out = []
    for _ in range(MAX_ITER):
        t_new = (1.0 + math.sqrt(1.0 + 4.0 * t * t)) / 2.0
        out.append((t - 1.0) / t_new)
        t = t_new
    return out


def _pack_p6(x):
    out = np.zeros((128, 6, 320), dtype=x.dtype)
    for ch in range(2):
        out[:, 3 * ch + 0] = x[ch, 0:128]
        out[:, 3 * ch + 1] = x[ch, 128:256]
        out[0:64, 3 * ch + 2] = x[ch, 256:320]
    return out


def _unpack_p6(p):
    out = np.zeros((2, 320, 320), dtype=p.dtype)
    for ch in range(2):
        out[ch, 0:128] = p[:, 3 * ch + 0]
        out[ch, 128:256] = p[:, 3 * ch + 1]
        out[ch, 256:320] = p[0:64, 3 * ch + 2]
    return out


def _host_consts():
    F, G = _dft_mats()
    Dd, Dg = _tv_mats()
    W1, W2, W3 = _haar_mat(320), _haar_mat(160), _haar_mat(80)
    f16 = np.float16
    return {
        "ftr": F.real.T.astype(f16), "fti": F.imag.T.astype(f16),
        "ftin": (-F.imag.T).astype(f16),
        "ifr": G.real.T.astype(f16), "ifi": G.imag.T.astype(f16),
        "ifin": (-G.imag.T).astype(f16), "ifrn": (-G.real.T).astype(f16),
        "ddt": Dd.T.astype(f16), "dgt": Dg.T.astype(f16),
        "w1t": W1.T.astype(f16), "w1h": (0.5 * W1).astype(f16),
        "w2t": W2.T.astype(f16), "w2h": (0.5 * W2).astype(f16),
        "w3t": W3.T.astype(f16), "w3h": (0.5 * W3).astype(f16),
        "idp": np.eye(128, dtype=f16), "idn": (-np.eye(128)).astype(f16),
    }


def _copy_segs(src_lay, dst_lay, nrows):
    out = {}
    for ch in (0, 1):
        def locate(lay, r):
            for (p0, p1, q, r0, r1) in lay[ch]:
                if r0 <= r < r1:
                    return p0 + (r - r0), q, r1 - r
            raise AssertionError(r)
        segs = []
        r = 0
        while r < nrows:
            sp, sq, sleft = locate(src_lay, r)
            dp, dq, dleft = locate(dst_lay, r)
            cnt = min(sleft, dleft, nrows - r)
            segs.append((sp, sq, dp, dq, cnt))
            r += cnt
        out[ch] = segs
    return out


SEG12 = _copy_segs(P6D, L2D, 160)
SEG23 = _copy_segs(L2D, L3D, 80)


def _build_nc():
    import concourse.bacc as bacc
    import concourse.tile as tile
    import concourse.mybir as mybir
    from contextlib import ExitStack

    dt = mybir.dt
    F32, F16 = dt.float32, dt.float16
    ALU = mybir.AluOpType
    AF = mybir.ActivationFunctionType

    s_tv = TAU * LAM_TV
    lam = LAM_TV
    eps_q = lam * lam * 1e-8
    lam1 = lam / s_tv
    eps1 = eps_q / (s_tv * s_tv)
    coeffs = _momentum_coeffs()
    lam_lvl = [LAM_WAV * (S2 ** (l + 1)) for l in range(LEVELS)]

    nc = bacc.Bacc("TRN2", target_bir_lowering=False, debug=False,
                   num_devices=NCORES)

    dr = {}
    for name in ("ftr", "fti", "ftin", "ifr", "ifi", "ifin", "ifrn",
                 "ddt", "dgt", "w1t", "w1h"):
        dr[name] = nc.dram_tensor(name, [320, 320], F16, kind="ExternalInput").ap()
    for name in ("w2t", "w2h"):
        dr[name] = nc.dram_tensor(name, [160, 160], F16, kind="ExternalInput").ap()
    for name in ("w3t", "w3h"):
        dr[name] = nc.dram_tensor(name, [80, 80], F16, kind="ExternalInput").ap()
    for name in ("idp", "idn"):
        dr[name] = nc.dram_tensor(name, [128, 128], F16, kind="ExternalInput").ap()
    for i in range(IMGS):
        dr[f"y{i}"] = nc.dram_tensor(f"y{i}", [128, 6, 320], F16, kind="ExternalInput").ap()
        dr[f"ym{i}"] = nc.dram_tensor(f"ym{i}", [128, 6, 320], F16, kind="ExternalInput").ap()
        dr[f"mk{i}"] = nc.dram_tensor(f"mk{i}", [128, 6, 320], F16, kind="ExternalInput").ap()
        dr[f"xo{i}"] = nc.dram_tensor(f"xo{i}", [128, 6, 320], F32, kind="ExternalOutput").ap()

    with ExitStack() as ctx:
        tc = ctx.enter_context(tile.TileContext(nc))
        st = ctx.enter_context(tc.tile_pool(name="state", bufs=1))
        psp = ctx.enter_context(tc.tile_pool(name="psp", bufs=8, space="PSUM"))

        def T(tag, shape, dtp):
            return st.tile(shape, dtp, tag=tag, name=tag)

        def PT():
            return psp.tile([128, 512], F32, tag="ps", name="pt")

        cv = {}
        for name in ("ftr", "fti", "ftin", "ifr", "ifi", "ifin", "ifrn",
                     "ddt", "dgt", "w1t", "w1h"):
            cv[name] = T("c_" + name, [128, 3, 320], F16)
        for name in ("w2t", "w2h"):
            cv[name] = T("c_" + name, [128, 2, 160], F16)
        for name in ("w3t", "w3h"):
            cv[name] = T("c_" + name, [128, 2, 80], F16)
        cv["idp"] = T("c_idp", [128, 1, 128], F16)
        cv["idn"] = T("c_idn", [128, 1, 128], F16)

        def load_const(name, lay):
            for (p0, p1, q, r0, r1) in lay:
                nc.sync.dma_start(cv[name][p0:p1, q, :], dr[name][r0:r1, :])

        for name in ("ftr", "fti", "ftin", "ifr", "ifi", "ifin", "ifrn",
                     "ddt", "dgt", "w1t", "w1h"):
            load_const(name, P6C)
        for name in ("w2t", "w2h"):
            load_const(name, L2C)
        for name in ("w3t", "w3h"):
            load_const(name, L3C)
        nc.sync.dma_start(cv["idp"][:, 0, :], dr["idp"][:])
        nc.sync.dma_start(cv["idn"][:, 0, :], dr["idn"][:])

        per_img = []
        for i in range(IMGS):
            per_img.append({
                "z": T(f"z{i}", [128, 6, 320], F16),
                "y16": T(f"y16_{i}", [128, 6, 320], F16),
                "c0": T(f"c0_{i}", [128, 6, 320], F16),
                "xA": T(f"xA{i}", [128, 6, 320], F32),
                "xB": T(f"xB{i}", [128, 6, 320], F32),
                "mk": T(f"msk{i}", [128, 6, 320], F16),
                "xc": T(f"xc{i}", [128, 6, 320], F32),
                "xcb": T(f"xcb{i}", [128, 6, 320], F16),
            })
        sbA = T("sbA", [128, 6, 320], F16)
        Km = T("Km", [128, 6, 320], F16)
        xtv32 = T("xtv32", [128, 6, 320], F32)
        wtmp = T("wtmp", [128, 6, 320], F32)
        w16 = T("w16", [128, 6, 320], F16)
        Y1 = T("Y1", [128, 6, 320], F16)
        Y2 = T("Y2", [128, 4, 160], F16)
        Y3 = T("Y3", [128, 4, 80], F16)
        L2t = T("L2t", [128, 4, 160], F16)
        L3t = T("L3t", [128, 4, 80], F16)
        qx = T("qx", [128, 6, 322], F32)
        qy = T("qy", [128, 6, 320], F16)
        vt = T("vt", [128, 6, 320], F16)
        tv1 = T("tv1", [128, 6, 320], F32)
        vsq = T("vsq", [128, 6, 320], F32)
        n2 = T("n2", [128, 6, 320], F32)
        rr = T("rr", [128, 6, 320], F32)
        sgn = T("sgn", [128, 6, 320], F16)

        nc.vector.memset(qx[:], 0.0)

        def fv(t, c0=0, c1=None):
            c1 = c1 if c1 is not None else t.shape[-1]
            return t[0:128, 0:6, c0:c1]

        def fvs(t, c0, c1, step):
            return t[0:128, 0:6, c0:c1:step]

        TT = nc.vector.tensor_tensor
        STT = nc.vector.scalar_tensor_tensor
        GTT = nc.gpsimd.tensor_tensor

        # ---------- matmul group emitters (one PSUM bank per M-tile) -------
        def fft_tile_group(data, terms, oc, mt, extras=()):
            mp0, mp1, mq, mr0, mr1 = mt
            pt = PT()
            mml = []
            for (dch, cname) in terms[oc]:
                cvt = cv[cname]
                for t in range(3):
                    dp0, dp1, dq, _, _ = P6D[dch][t]
                    cp0, cp1, cq, _, _ = P6C[t]
                    mml.append((data[dp0:dp1, dq, mr0:mr1],
                                cvt[cp0:cp1, cq, 0:320]))
            for (iname, x) in extras:
                msz = mp1 - mp0
                mml.append((cv[iname][0:msz, 0, 0:msz],
                            x[mp0:mp1, mq, 0:320]))
            n = len(mml)
            for idx, (l, r) in enumerate(mml):
                nc.tensor.matmul(pt[mp0:mp1, 0:320], l, r,
                                 start=(idx == 0), stop=(idx == n - 1))
            return pt

        def emit_fft_stage(data, terms, consumer, extras=None):
            for oc in (0, 1):
                for mt in P6D[oc]:
                    pt = fft_tile_group(data, terms, oc, mt,
                                        extras(oc, mt) if extras else ())
                    consumer(oc, mt, pt)
                    yield

        def emit_mm_left(cname, data, dlay, clay, ncols, consumer):
            for ch in (0, 1):
                dts = dlay[ch]
                n = len(dts)
                for mt in dts:
                    mp0, mp1, mq, mr0, mr1 = mt
                    pt = PT()
                    for t in range(n):
                        dp0, dp1, dq, _, _ = dts[t]
                        cp0, cp1, cq, _, _ = clay[t]
                        nc.tensor.matmul(
                            pt[mp0:mp1, 0:ncols],
                            cv[cname][cp0:cp1, cq, mr0:mr1],
                            data[dp0:dp1, dq, 0:ncols],
                            start=(t == 0), stop=(t == n - 1))
                    consumer(ch, mt, pt)
                    yield

        FWDT = {0: [(0, "ftr"), (1, "ftin")], 1: [(0, "fti"), (1, "ftr")]}
        INVT = {0: [(0, "ifr"), (1, "ifin")], 1: [(0, "ifi"), (1, "ifr")]}
        INV2T = {0: [(0, "ifrn"), (1, "ifi")], 1: [(0, "ifin"), (1, "ifrn")]}

        def drain(gen):
            for _ in gen:
                pass

        # ---------- init: x0 = ifft2(y); c0 = ifft2(mask*y) ----------
        for i in range(IMGS):
            im = per_img[i]
            nc.sync.dma_start(im["y16"][:], dr[f"y{i}"][:])
            nc.sync.dma_start(im["mk"][:], dr[f"mk{i}"][:])

            def c_s1(oc, mt, pt):
                mp0, mp1, mq, _, _ = mt
                nc.scalar.copy(sbA[mp0:mp1, mq, :], pt[mp0:mp1, 0:320])

            def c_x0(oc, mt, pt, im=im):
                mp0, mp1, mq, _, _ = mt
                nc.scalar.copy(im["xA"][mp0:mp1, mq, :], pt[mp0:mp1, 0:320])
                nc.vector.tensor_copy(im["z"][mp0:mp1, mq, :], pt[mp0:mp1, 0:320])

            def c_c0(oc, mt, pt, im=im):
                mp0, mp1, mq, _, _ = mt
                nc.scalar.copy(im["c0"][mp0:mp1, mq, :], pt[mp0:mp1, 0:320])

            drain(emit_fft_stage(im["y16"], INVT, c_s1))
            drain(emit_fft_stage(sbA, INVT, c_x0))
            nc.sync.dma_start(Km[:], dr[f"ym{i}"][:])
            drain(emit_fft_stage(Km, INVT, c_s1))
            drain(emit_fft_stage(sbA, INVT, c_c0))

        # ---------- phase A: data-fidelity gradient step -> xc ----------
        def emitA(i):
            im = per_img[i]

            def c_s1(oc, mt, pt):
                mp0, mp1, mq, _, _ = mt
                nc.scalar.copy(sbA[mp0:mp1, mq, :], pt[mp0:mp1, 0:320])

            yield from emit_fft_stage(im["z"], FWDT, c_s1)

            def ex_s2(oc, mt, im=im):
                return (("idn", im["y16"]),)

            def c_s2(oc, mt, pt, im=im):
                mp0, mp1, mq, _, _ = mt
                TT(Km[mp0:mp1, mq, :], pt[mp0:mp1, 0:320],
                   im["mk"][mp0:mp1, mq, :], ALU.mult)

            yield from emit_fft_stage(sbA, FWDT, c_s2, ex_s2)
            yield from emit_fft_stage(Km, INVT, c_s1)

            def ex_is2(oc, mt, im=im):
                # Km already includes -mask*y (folded in fwd stage2), so the
                # inverse gives -g directly; just add z: psum = z - g = z_step
                return (("idp", im["z"]),)

            def c_is2(oc, mt, pt, im=im):
                mp0, mp1, mq, _, _ = mt
                nc.scalar.copy(im["xc"][mp0:mp1, mq, :], pt[mp0:mp1, 0:320])
                nc.scalar.copy(im["xcb"][mp0:mp1, mq, :], pt[mp0:mp1, 0:320])

            yield from emit_fft_stage(sbA, INV2T, c_is2, ex_is2)

        # ---------- phase B: TV prox + DWT correction + momentum ----------
        def emitB(i, k):
            im = per_img[i]
            xc, xcb = im["xc"], im["xcb"]
            xold = im["xA"] if k % 2 == 0 else im["xB"]
            xnew = im["xB"] if k % 2 == 0 else im["xA"]

            if SKIP_TV:
                nc.vector.tensor_copy(fv(xtv32), fv(xc))
                yield
            else:
                # ----- it 0: p1 = Pi(tau*grad(xc)) in q = lam*p units -----
                def c_g0(ch, mt, pt):
                    mp0, mp1, mq, _, _ = mt
                    nc.scalar.square(vsq[mp0:mp1, mq, :], pt[mp0:mp1, 0:320])
                    nc.vector.tensor_copy(qy[mp0:mp1, mq, :], pt[mp0:mp1, 0:320])

                yield from emit_mm_left("dgt", xcb, P6D, P6C, 320, c_g0)
                TT(fv(qx, 2, 321), fv(xc, 1, 320), fv(xc, 0, 319), ALU.subtract)
                nc.scalar.square(fv(tv1), fv(qx, 2, 322))
                yield
                GTT(fv(n2), fv(tv1), fv(vsq), ALU.add)
                nc.scalar.activation(fv(rr), fv(n2), AF.Abs_reciprocal_sqrt,
                                     scale=1.0 / (lam * lam))
                yield
                STT(fv(qx, 2, 321), fv(rr, 0, 319), s_tv, fv(qx, 2, 321),
                    ALU.min, ALU.mult)
                STT(fv(qy), fv(rr), s_tv, fv(qy), ALU.min, ALU.mult)
                yield

                for it in range(1, TV_ITERS):
                    STT(fv(tv1), fv(qx, 2, 322), -1.0, fv(xc), ALU.mult, ALU.add)
                    GTT(fv(tv1), fv(tv1), fv(qx, 1, 321), ALU.add)
                    yield

                    def c_vt(ch, mt, pt):
                        mp0, mp1, mq, _, _ = mt
                        TT(vt[mp0:mp1, mq, :], tv1[mp0:mp1, mq, :],
                           pt[mp0:mp1, 0:320], ALU.subtract)

                    yield from emit_mm_left("ddt", qy, P6D, P6C, 320, c_vt)

                    def c_gy(ch, mt, pt):
                        mp0, mp1, mq, _, _ = mt
                        STT(qy[mp0:mp1, mq, :], pt[mp0:mp1, 0:320], s_tv,
                            qy[mp0:mp1, mq, :], ALU.mult, ALU.add)

                    gy_gen = emit_mm_left("dgt", vt, P6D, P6C, 320, c_gy)
                    STT(fv(qx, 2, 321), fv(vt, 1, 320), s_tv, fv(qx, 2, 321),
                        ALU.mult, ALU.add)
                    STT(fv(qx, 2, 321), fv(vt, 0, 319), -s_tv, fv(qx, 2, 321),
                        ALU.mult, ALU.add)
                    yield from gy_gen
                    nc.scalar.square(fv(tv1), fv(qx, 2, 322))
                    nc.scalar.square(fv(vsq), fv(qy))
                    yield
                    GTT(fv(n2), fv(tv1), fv(vsq), ALU.add)
                    nc.scalar.activation(fv(rr), fv(n2), AF.Abs_reciprocal_sqrt,
                                         scale=1.0 / (lam * lam))
                    yield
                    STT(fv(qx, 2, 321), fv(rr, 0, 319), 1.0, fv(qx, 2, 321),
                        ALU.min, ALU.mult)
                    STT(fv(qy), fv(rr), 1.0, fv(qy), ALU.min, ALU.mult)
                    yield

                # ----- x_tv = xc - lam*div(p) -----
                STT(fv(xtv32), fv(qx, 2, 322), -1.0, fv(xc), ALU.mult, ALU.add)
                GTT(fv(xtv32), fv(xtv32), fv(qx, 1, 321), ALU.add)
                yield

                def c_fin(ch, mt, pt):
                    mp0, mp1, mq, _, _ = mt
                    TT(xtv32[mp0:mp1, mq, :], xtv32[mp0:mp1, mq, :],
                       pt[mp0:mp1, 0:320], ALU.subtract)

                yield from emit_mm_left("ddt", qy, P6D, P6C, 320, c_fin)

            if SKIP_DWT:
                nc.vector.tensor_copy(fv(xnew), fv(xtv32))
                yield
            else:
                # ----- DWT forward -----
                TT(fv(w16, 0, 160), fvs(xtv32, 0, 320, 2),
                   fvs(xtv32, 1, 320, 2), ALU.add)
                TT(fv(w16, 160, 320), fvs(xtv32, 0, 320, 2),
                   fvs(xtv32, 1, 320, 2), ALU.subtract)
                yield

                def c_y1(ch, mt, pt):
                    mp0, mp1, mq, _, _ = mt
                    nc.scalar.copy(Y1[mp0:mp1, mq, :], pt[mp0:mp1, 0:320])

                yield from emit_mm_left("w1t", w16, P6D, P6C, 320, c_y1)
                for ch in (0, 1):
                    for (sp, sq, dp, dq, cnt) in SEG12[ch]:
                        GTT(L2t[dp:dp + cnt, dq, 0:80],
                            Y1[sp:sp + cnt, sq, 0:160:2],
                            Y1[sp:sp + cnt, sq, 1:160:2], ALU.add)
                        GTT(L2t[dp:dp + cnt, dq, 80:160],
                            Y1[sp:sp + cnt, sq, 0:160:2],
                            Y1[sp:sp + cnt, sq, 1:160:2], ALU.subtract)
                yield

                def c_y2(ch, mt, pt):
                    mp0, mp1, mq, _, _ = mt
                    nc.scalar.copy(Y2[mp0:mp1, mq, :], pt[mp0:mp1, 0:160])

                yield from emit_mm_left("w2t", L2t, L2D, L2C, 160, c_y2)
                for ch in (0, 1):
                    for (sp, sq, dp, dq, cnt) in SEG23[ch]:
                        GTT(L3t[dp:dp + cnt, dq, 0:40],
                            Y2[sp:sp + cnt, sq, 0:80:2],
                            Y2[sp:sp + cnt, sq, 1:80:2], ALU.add)
                        GTT(L3t[dp:dp + cnt, dq, 40:80],
                            Y2[sp:sp + cnt, sq, 0:80:2],
                            Y2[sp:sp + cnt, sq, 1:80:2], ALU.subtract)
                yield

                def c_y3(ch, mt, pt):
                    mp0, mp1, mq, _, _ = mt
                    nc.scalar.copy(Y3[mp0:mp1, mq, :], pt[mp0:mp1, 0:80])

                yield from emit_mm_left("w3t", L3t, L3D, L3C, 80, c_y3)

                # delta = soft(Y)-Y = -sign(Y)*min(|Y|,lam); LL3 zeroed
                def delta(views, lam_l, m_views, s_views):
                    for i2 in range(len(views)):
                        nc.scalar.activation(m_views[i2], views[i2], AF.Abs)
                        nc.scalar.activation(s_views[i2], views[i2], AF.Sign)
                        nc.vector.tensor_scalar(m_views[i2], m_views[i2],
                                                lam_l, -1.0, ALU.min, ALU.mult)
                        nc.vector.tensor_tensor(views[i2], m_views[i2],
                                                s_views[i2], ALU.mult)

                delta([Y3[0:64, 0:4:2, :], Y3[0:16, 1:4:2, :]], lam_lvl[2],
                      [w16[0:64, 0:4:2, 0:80], w16[0:16, 1:4:2, 0:80]],
                      [sgn[0:64, 0:4:2, 0:80], sgn[0:16, 1:4:2, 0:80]])
                nc.vector.memset(Y3[0:40, 0:4:2, 0:40], 0.0)
                yield
                delta([Y2[0:128, 0:4:2, :], Y2[0:32, 1:4:2, :]], lam_lvl[1],
                      [w16[0:128, 0:4:2, 0:160], w16[0:32, 1:4:2, 0:160]],
                      [sgn[0:128, 0:4:2, 0:160], sgn[0:32, 1:4:2, 0:160]])
                yield
                delta([fv(Y1)], lam_lvl[0], [fv(w16)], [fv(sgn)])
                yield

                # ----- inverse on deltas -> correction -----
                def c_l3(ch, mt, pt):
                    mp0, mp1, mq, _, _ = mt
                    nc.scalar.copy(L3t[mp0:mp1, mq, :], pt[mp0:mp1, 0:80])

                yield from emit_mm_left("w3h", Y3, L3D, L3C, 80, c_l3)
                for ch in (0, 1):
                    for (sp, sq, dp, dq, cnt) in SEG23[ch]:
                        GTT(Y2[sp:sp + cnt, sq, 0:80:2],
                            L3t[dp:dp + cnt, dq, 0:40],
                            L3t[dp:dp + cnt, dq, 40:80], ALU.add)
                        GTT(Y2[sp:sp + cnt, sq, 1:80:2],
                            L3t[dp:dp + cnt, dq, 0:40],
                            L3t[dp:dp + cnt, dq, 40:80], ALU.subtract)
                yield

                def c_l2(ch, mt, pt):
                    mp0, mp1, mq, _, _ = mt
                    nc.scalar.copy(L2t[mp0:mp1, mq, :], pt[mp0:mp1, 0:160])

                yield from emit_mm_left("w2h", Y2, L2D, L2C, 160, c_l2)
                for ch in (0, 1):
                    for (sp, sq, dp, dq, cnt) in SEG12[ch]:
                        GTT(Y1[sp:sp + cnt, sq, 0:160:2],
                            L2t[dp:dp + cnt, dq, 0:80],
                            L2t[dp:dp + cnt, dq, 80:160], ALU.add)
                        GTT(Y1[sp:sp + cnt, sq, 1:160:2],
                            L2t[dp:dp + cnt, dq, 0:80],
                            L2t[dp:dp + cnt, dq, 80:160], ALU.subtract)
                yield

                def c_corr(ch, mt, pt):
                    mp0, mp1, mq, _, _ = mt
                    nc.scalar.copy(wtmp[mp0:mp1, mq, :], pt[mp0:mp1, 0:320])

                yield from emit_mm_left("w1h", Y1, P6D, P6C, 320, c_corr)
                # x_new = x_tv + correction (even/odd interleave)
                TT(fvs(xnew, 0, 320, 2), fv(wtmp, 0, 160),
                   fv(wtmp, 160, 320), ALU.add)
                TT(fvs(xnew, 1, 320, 2), fv(wtmp, 0, 160),
                   fv(wtmp, 160, 320), ALU.subtract)
                yield
                GTT(fvs(xnew, 0, 320, 2), fvs(xnew, 0, 320, 2),
                    fvs(xtv32, 0, 320, 2), ALU.add)
                TT(fvs(xnew, 1, 320, 2), fvs(xnew, 1, 320, 2),
                   fvs(xtv32, 1, 320, 2), ALU.add)
                yield

            # ----- momentum -----
            if k < MAX_ITER - 1:
                GTT(fv(wtmp), fv(xnew), fv(xold), ALU.subtract)
                yield
                STT(fv(im["z"]), fv(wtmp), coeffs[k], fv(xnew),
                    ALU.mult, ALU.add)
                yield

        def interleave(*gens):
            live = [g for g in gens if g is not None]
            while live:
                nxt = []
                for g in live:
                    try:
                        next(g)
                        nxt.append(g)
                    except StopIteration:
                        continue
                live = nxt

        # ---------- software-pipelined FISTA ----------
        # A(i) computes xc_i for the upcoming B(i, .) from z_i; B's TV/DWT
        # (DVE/ACT/GPSIMD-heavy) overlaps the other image's A (PE-heavy).
        drain(emitA(0))
        for k in range(MAX_ITER):
            interleave(emitB(0, k), emitA(1))
            interleave(emitB(1, k), emitA(0) if k + 1 < MAX_ITER else None)

        fin = "xB" if (MAX_ITER - 1) % 2 == 0 else "xA"
        for i in range(IMGS):
            nc.sync.dma_start(dr[f"xo{i}"][:], per_img[i][fin][:])

    nc.compile()
    return nc


_NC = None


def _get_nc():
    global _NC
    if _NC is None:
        _NC = _build_nc()
    return _NC


def _in_maps(y, mask, c):
    in_maps = []
    for core in range(NCORES):
        m = dict(c)
        for i in range(IMGS):
            b = core * IMGS + i
            mpair = np.broadcast_to(mask[b], (2, 320, 320))
            m[f"y{i}"] = _pack_p6(y[b]).astype(np.float16)
            m[f"ym{i}"] = _pack_p6((mask[b] * y[b])).astype(np.float16)
            m[f"mk{i}"] = _pack_p6(mpair).astype(np.float16)
        in_maps.append(m)
    return in_maps


def kernel(y, mask):
    from concourse.bass_utils import run_bass_kernel_spmd

    y = np.asarray(y, dtype=np.float32)
    mask = np.asarray(mask, dtype=np.float32)
    c = _host_consts()
    nc = _get_nc()
    in_maps = _in_maps(y, mask, c)

    res = run_bass_kernel_spmd(nc, in_maps, list(range(NCORES)))
    out = np.zeros((B, 2, H, W), dtype=np.float32)
    for core in range(NCORES):
        for i in range(IMGS):
            out[core * IMGS + i] = _unpack_p6(res.results[core][f"xo{i}"])
    return out
